# revision 8
# baseline (speedup 1.0000x reference)
"""Trainium2 Bass kernel for the segment-reduce clustering-loss problem.

Math: with per-class sums S_c = sum_{i: l_i=c} x_i, counts n_c, and
total sum-of-squares Q = sum_i ||x_i||^2, everything follows:
    mu_c       = S_c / n_c
    compactness = Q - sum_c n_c ||mu_c||^2
    separation  = sum_c n_c ||mu_c||^2 - N ||mu_g||^2,  mu_g = (sum_c S_c)/N
So the device only needs ONE streaming pass over features computing
(S, n, Q) partials per core; the host combines the tiny partials.

Device strategy per core (data-parallel over N, 8 cores):
  - rows are assigned row = p*BPP + b  ->  partition p, block b so that both
    the labels DMA and the per-block feature DMAs are fully contiguous.
  - per 128-row block: one-hot(labels) built on DVE, segment sums via a
    float32r one-hot matmul accumulated in PSUM (16 x 512).
  - sum-of-squares via ScalarE activation(Square, accum_out) and DVE
    tensor_tensor_reduce, alternating to split the load.
"""

import numpy as np

import concourse.bass as bass
import concourse.tile as tile
from concourse import mybir
from concourse.bass_utils import run_bass_kernel_spmd

N, D, C = 262144, 512, 16
NCORES = 8
R = N // NCORES            # rows per core = 32768
P = 128                    # SBUF partitions
BPP = R // P               # blocks per core = 256
F = 8                      # blocks per feature DMA group
NG = BPP // F              # DMA groups = 32

_F32 = mybir.dt.float32
_F32R = mybir.dt.float32r
_I32 = mybir.dt.int32


def _patched_drain_and_barrier(self, tick_clock, wait_clock):
    """The walrus build here encodes at most ONE sync-wait on a Drain
    (CoreV3 setupSyncWait: 'Too many sync wait commands'). Split the
    kernel-tail drain's waits across chained single-wait drains."""
    from concourse.vector_clock import ScopedClock

    drain_inst = self.nc.sync.drain()
    wait_clock.add_sem_waits(
        drain_inst.ins, ScopedClock({None: tick_clock.global_clock})
    )
    si = drain_inst.ins.sync_info
    if si is not None and len(si.on_wait) > 1:
        waits = list(si.on_wait)
        si.on_wait = waits[:1]
        for w in waits[1:]:
            d2 = self.nc.sync.drain()
            d2.ins.sync_info = mybir.SyncInfo(on_wait=[w], on_update=[])
    self.nc.all_engine_barrier()
    assert self.sems is not None
    popped = self.nc._tile_sem_poison_stack.pop()
    assert popped is self._sem_poison
    # clear_and_free_semaphores emits an EVENT_SEMAPHORE_RANGE_CLEAR InstISA
    # this walrus can't encode ('ISA wrong length'); use the legacy
    # drain(is_reset_sema, range) mechanism instead.
    sems = list(self.sems.allocated().values())
    if sems:
        from concourse.bass import SemaphoreHandle

        sem_nums = sorted(
            s.num if isinstance(s, SemaphoreHandle) else s for s in sems
        )
        ranges = []
        lo = prev = sem_nums[0]
        for s in sem_nums[1:]:
            if s == prev + 1:
                prev = s
            else:
                ranges.append(range(lo, prev + 1))
                lo = prev = s
        ranges.append(range(lo, prev + 1))
        for rng in ranges:
            self.nc.gpsimd.drain(semaphore_range=rng)
        self.nc._state.prepend_free_semaphores(sem_nums)
        for poison_set in self.nc._tile_sem_poison_stack:
            poison_set.update(sem_nums)
    self.nc.all_engine_barrier()


tile.TileContext._drain_and_barrier = _patched_drain_and_barrier


def _split_multi_waits(nc):
    """This walrus build encodes at most one sync-wait per instruction.
    Hoist extra waits onto single-wait NoOp carriers placed just before the
    owning instruction on the same engine (identical blocking semantics)."""
    n_new = 0
    for fn in nc.m.functions:
        for bb in fn.blocks:
            new_insts = []
            for inst in bb.instructions:
                si = inst.sync_info
                if si is not None and len(si.on_wait) > 1:
                    waits = list(si.on_wait)
                    for i, w in enumerate(waits[:-1]):
                        nop = mybir.InstNoOp(
                            name=f"{inst.name}-wsplit{i}",
                            engine=inst.engine,
                            sync_info=mybir.SyncInfo(on_wait=[w], on_update=[]),
                            bass_nofuse=True,
                        )
                        nc.register_instruction(nop, overwrite=True)
                        new_insts.append(nop)
                        n_new += 1
                    si.on_wait = waits[-1:]
                new_insts.append(inst)
            bb.instructions[:] = new_insts
    return n_new


def build_program(r=R):
    bpp = r // P
    ng = bpp // F
    assert bpp * P == r and ng * F == bpp

    nc = bass.Bass()
    feat = nc.dram_tensor("features", [r, D], _F32R, kind="ExternalInput")
    lab = nc.dram_tensor("labels", [r], _I32, kind="ExternalInput")
    out_sums = nc.dram_tensor("out_sums", [C, D], _F32, kind="ExternalOutput")
    out_misc = nc.dram_tensor("out_misc", [P, C + 1], _F32, kind="ExternalOutput")

    # row = p*bpp + (g*F + f)  ->  partition p, group g, sub-block f
    feat_r = feat.rearrange("(p g f) d -> g p f d", p=P, g=ng, f=F)
    lab_r = lab.rearrange("(p b) -> p b", p=P)

    with tile.TileContext(nc) as tc:
        with (
            tc.tile_pool(name="singles", bufs=1) as singles,
            tc.tile_pool(name="feats", bufs=3) as feats,
            tc.tile_pool(name="scratch", bufs=1) as scratch,
            tc.tile_pool(name="psum", bufs=1, space="PSUM") as psum_pool,
        ):
            lab_i = singles.tile([P, bpp], _I32)
            nc.sync.dma_start(out=lab_i, in_=lab_r)
            lab_f = singles.tile([P, bpp], _F32)
            nc.vector.tensor_copy(lab_f, lab_i)

            # every partition holds the row [0, 1, ..., C-1]
            iota_f = singles.tile([P, C], _F32)
            for c in range(C):
                nc.vector.memset(iota_f[:, c : c + 1], float(c))

            # one-hot for every block: onehot[p, b, c] = (labels[p, b] == c)
            onehot = singles.tile([P, bpp, C], _F32R)
            for b in range(bpp):
                nc.vector.tensor_scalar(
                    out=onehot[:, b, :],
                    in0=iota_f,
                    scalar1=lab_f[:, b : b + 1],
                    scalar2=None,
                    op0=mybir.AluOpType.is_equal,
                )

            sums_psum = psum_pool.tile([C, D], _F32)
            sq_accs = singles.tile([P, ng], _F32)

            for g in range(ng):
                ft = feats.tile([P, F, D], _F32R, tag="ft")
                nc.sync.dma_start(out=ft, in_=feat_r[g])
                for f_i in range(F):
                    b = g * F + f_i
                    nc.tensor.matmul(
                        sums_psum[:],
                        onehot[:, b, :],
                        ft[:, f_i, :],
                        start=(b == 0),
                        stop=(b == bpp - 1),
                    )
                sq = scratch.tile([P, F, D], _F32, tag="sq_a")
                nc.scalar.activation(
                    out=sq,
                    in_=ft.bitcast(_F32),
                    func=mybir.ActivationFunctionType.Square,
                    accum_out=sq_accs[:, g : g + 1],
                )

            misc = singles.tile([P, C + 1], _F32)
            nc.vector.tensor_reduce(
                out=misc[:, 0:C],
                in_=onehot.transpose([0, 2, 1]),
                axis=mybir.AxisListType.X,
                op=mybir.AluOpType.add,
            )
            nc.vector.tensor_reduce(
                out=misc[:, C : C + 1],
                in_=sq_accs,
                axis=mybir.AxisListType.X,
                op=mybir.AluOpType.add,
            )
            sums_sb = singles.tile([C, D], _F32)
            nc.vector.tensor_copy(sums_sb, sums_psum)
            nc.sync.dma_start(out=out_sums[:], in_=sums_sb)
            nc.sync.dma_start(out=out_misc[:], in_=misc)

    _split_multi_waits(nc)
    return nc


_CACHE = {}


def _get_program():
    if "nc" not in _CACHE:
        _CACHE["nc"] = build_program()
    return _CACHE["nc"]


def run_device(features, labels, trace=False):
    """Run the SPMD bass kernel on 8 cores; returns (per-core results, exec_time_ns)."""
    nc = _get_program()
    in_maps = []
    for c in range(NCORES):
        sl = slice(c * R, (c + 1) * R)
        in_maps.append(
            {
                "features": np.ascontiguousarray(features[sl]),
                "labels": np.ascontiguousarray(labels[sl]),
            }
        )
    out = run_bass_kernel_spmd(nc, in_maps, list(range(NCORES)), trace=trace)
    return out.results, out.exec_time_ns


def combine(results):
    """All-reduce the per-core partials and finish the O(C*D) math on host."""
    sums = np.zeros((C, D), np.float64)
    counts = np.zeros((C,), np.float64)
    sumsq = 0.0
    for r_ in results:
        sums += r_["out_sums"].astype(np.float64)
        misc = r_["out_misc"].astype(np.float64)
        counts += misc[:, 0:C].sum(axis=0)
        sumsq += float(misc[:, C].sum())
    means = sums / counts[:, None]
    gmean = sums.sum(axis=0) / N
    n_mu2 = float((counts * (means * means).sum(axis=1)).sum())
    compactness = sumsq - n_mu2
    separation = n_mu2 - N * float((gmean * gmean).sum())
    return (
        np.array([compactness], np.float32),
        np.array([separation], np.float32),
        means[:, None, :].astype(np.float32),
    )


def kernel(features, labels, class_num):
    features = np.ascontiguousarray(np.asarray(features), dtype=np.float32)
    labels = np.ascontiguousarray(np.asarray(labels).astype(np.int32, copy=False))
    assert int(class_num) == C and features.shape == (N, D) and labels.shape == (N,)
    results, _ = run_device(features, labels, trace=False)
    return combine(results)
